# revision 1
# baseline (speedup 1.0000x reference)
"""Transformer block (pre-norm attn + MLP) on 8 NeuronCores, data-parallel
over batch. Full inputs in, full outputs out; each core runs one batch
element x[i] : [1024, 768] through an identical Bass/Tile kernel.

v2: single fully-pipelined emission (no phase barriers).

Host-side exact refactoring (as v1):
  - LN gains fold into the following matmul weights: diag(g) @ W.
  - LN biases fold into: per-column bias on q/k, b_proj_eff (v-bias passes
    through softmax additively), b_fc1_eff.
  - w_proj rows re-laid-out head-aligned: block h rows 1..96 (row 0 pairs
    with the attention colsum row; zero).
  - x is cast to bf16 on host (halves DMA, 2x bn_stats rate); weights bf16.

Device dataflow per core (emission = PE execution order):
  LN1 per token tile: bn_stats + Newton rsqrt (DVE only; the scalar engine
    stays in the exp table-set all the way to the MLP so there is exactly
    one table swap, exp->gelu) -> h bf16 -> PE-transpose -> h_fm [C, N].
  qkv/attention software-pipelined per head: qk MMs (96-part psum) ->
    scores S^T = k.T q (contraction 96, no zero-padding) -> exp on ACT ->
    PV with leading-ones v column (row 0 = colsum) -> reciprocal_approx
    + gpsimd partition-broadcast -> o_fm.  v-generation and proj/LN2
    transposes are interleaved as PE filler under the exp latency.
  proj: x1 = x + o @ w_proj (+b folded into x beforehand), in-place bf16.
  LN2 per tile (Newton rstd) -> transpose -> h2_fm.
  MLP: fc1 per token-half (free=512) -> gelu -> g bf16; fc2 per quarter
    accumulating all 24 ff tiles in PSUM; +x1 -> y.  wfc1/wfc2 are loaded
    whole, once, into SBUF space freed by wqkv/h_fm.
"""
import numpy as np
import ml_dtypes

import concourse.bass as bass
from concourse import bacc, mybir
from concourse.bass_utils import run_bass_kernel_spmd
from concourse.masks import make_identity
from concourse.tile import TileContext

P = 128
N = 1024          # tokens per core (batch element)
C = 768           # model dim
H = 8             # heads
DH = C // H       # 96
DFF = 4 * C       # 3072
NT = N // P       # 8 token tiles
KT = C // P       # 6 feature tiles
FFT = DFF // P    # 24 ff tiles
NH = 2            # halves of the token axis for attention
NC_ = N // NH     # 512
EPS = 1e-5
SCALE = DH ** -0.5
VW = DH           # per-head v width (plus a leading ones column)

F32 = mybir.dt.float32
BF16 = mybir.dt.bfloat16
MULT = mybir.AluOpType.mult
ADD = mybir.AluOpType.add

_CACHED = {}


def build(taps=()):
    nc = bacc.Bacc("TRN2", debug=False)

    x_d = nc.dram_tensor("x_bf", [N, C], BF16, kind="ExternalInput")
    wqkv_d = nc.dram_tensor("w_qkv_e", [C, 3 * C], BF16, kind="ExternalInput")
    wproj_d = nc.dram_tensor("w_proj_p", [H * P, C], BF16, kind="ExternalInput")
    wfc1_d = nc.dram_tensor("w_fc1_e", [C, DFF], BF16, kind="ExternalInput")
    wfc2_d = nc.dram_tensor("w_fc2", [DFF, C], BF16, kind="ExternalInput")
    qkb_d = nc.dram_tensor("qk_bias", [P, 2 * H], F32, kind="ExternalInput")
    bp_d = nc.dram_tensor("b_proj_e", [C], BF16, kind="ExternalInput")
    bf1_d = nc.dram_tensor("b_fc1_e", [DFF], F32, kind="ExternalInput")
    bf2_d = nc.dram_tensor("b_fc2", [C], BF16, kind="ExternalInput")
    y_d = nc.dram_tensor("y", [N, C], F32, kind="ExternalOutput")

    tap_d = {}
    for name, shape, dt in [
        ("h_fm", [C, N], BF16),
        ("q_fm", [H * P, N], BF16),
        ("k_fm", [H * P, N], BF16),
        ("o_fm", [H * P, N], BF16),
        ("x1", [N, C], BF16),
        ("h2_fm", [C, N], BF16),
    ]:
        if name in taps:
            tap_d[name] = nc.dram_tensor(
                "tap_" + name, shape, dt, kind="ExternalOutput"
            )

    def bcast_row(dram_t, width):
        return bass.AP(tensor=dram_t, offset=0, ap=[[0, P], [1, width]])

    with TileContext(nc) as tc:
        # ---------------- SBUF pools, LEFT stack (bottom -> top) --------
        consts = tc.alloc_tile_pool(name="consts", bufs=1, side="left")
        xpool = tc.alloc_tile_pool(name="xpool", bufs=1, side="left")
        wprojp = tc.alloc_tile_pool(name="wprojp", bufs=1, side="left")
        h2p = tc.alloc_tile_pool(name="h2p", bufs=1, side="left")
        lnscr = tc.alloc_tile_pool(name="lnscr", bufs=2, side="left")
        hfmp = tc.alloc_tile_pool(name="hfmp", bufs=1, side="left")
        wqkvp = tc.alloc_tile_pool(name="wqkvp", bufs=1, side="left")

        # ---------------- SBUF pools, RIGHT stack -----------------------
        opool = tc.alloc_tile_pool(name="opool", bufs=1, side="right")
        rrow = tc.alloc_tile_pool(name="rrow", bufs=1, side="right")
        vpool = tc.alloc_tile_pool(name="vpool", bufs=1, side="right")
        epool = tc.alloc_tile_pool(name="epool", bufs=4, side="right")
        qpool = tc.alloc_tile_pool(name="qpool", bufs=1, side="right")
        kpool = tc.alloc_tile_pool(name="kpool", bufs=1, side="right")

        # ---------------- PSUM pools ------------------------------------
        work1 = tc.alloc_tile_pool(name="work1", bufs=2, space="PSUM")
        tpps = tc.alloc_tile_pool(name="tpps", bufs=2, space="PSUM")
        sps = tc.alloc_tile_pool(name="sps", bufs=2, space="PSUM")

        # ---------------- constants ------------------------------------
        ident = consts.tile([P, P], BF16)
        make_identity(nc, ident)
        eps_t = consts.tile([P, 1], F32)
        nc.vector.memset(eps_t, EPS)
        dum = consts.tile([1, 1], F32)
        qkb = consts.tile([P, 2 * H], F32)
        bf1c = consts.tile([P, FFT], F32)
        bpb = consts.tile([P, C], BF16)
        bf2b = consts.tile([P, C], BF16)

        # ---------------- big tiles + DMAs ------------------------------
        # spread across engine queues: each queue moves ~130 GB/s, so the
        # startup loads (x then wq/wk) go wide, ordered by first consumer
        x_tok = xpool.tile([P, NT, C], BF16)
        xr = x_d.rearrange("(nt p) c -> p nt c", p=P)
        wqkv = wqkvp.tile([P, KT, 3 * C], BF16)
        wr = wqkv_d.rearrange("(kt p) o -> p kt o", p=P)
        # strict priority via per-queue heads: every queue starts with an
        # x chunk, then q weights, then k, then v — so the DMA fabric
        # (~260 GB/s shared) always serves the next-needed tensor first.
        # The per-partition bias rows load compactly (1 partition) and are
        # partition-broadcast on-chip: a [0,P]-stride DMA costs 128 tiny
        # descriptors and stalls its whole queue for ~10us.
        nc.sync.dma_start(x_tok[:, 0:2, :], xr[:, 0:2, :])
        nc.gpsimd.dma_start(x_tok[:, 2:4, :], xr[:, 2:4, :])
        nc.scalar.dma_start(x_tok[:, 4:NT, :], xr[:, 4:NT, :])
        nc.sync.dma_start(wqkv[:, :, 0:384], wr[:, :, 0:384])
        nc.gpsimd.dma_start(wqkv[:, :, 384:C], wr[:, :, 384:C])
        nc.scalar.dma_start(qkb[:], qkb_d[:, :])
        nc.sync.dma_start(wqkv[:, :, C:C + 384], wr[:, :, C:C + 384])
        nc.gpsimd.dma_start(wqkv[:, :, C + 384:2 * C], wr[:, :, C + 384:2 * C])
        nc.sync.dma_start(wqkv[:, :, 2 * C:2 * C + 384], wr[:, :, 2 * C:2 * C + 384])
        nc.gpsimd.dma_start(wqkv[:, :, 2 * C + 384:3 * C], wr[:, :, 2 * C + 384:])

        brow1 = consts.tile([1, C], BF16)
        brow2 = consts.tile([1, C], BF16)
        nc.scalar.dma_start(
            brow1[0:1, :], bass.AP(tensor=bp_d, offset=0, ap=[[0, 1], [1, C]])
        )
        nc.scalar.dma_start(
            brow2[0:1, :], bass.AP(tensor=bf2_d, offset=0, ap=[[0, 1], [1, C]])
        )
        nc.gpsimd.partition_broadcast(bpb[:, :], brow1[0:1, :])
        nc.gpsimd.partition_broadcast(bf2b[:, :], brow2[0:1, :])

        wproj = wprojp.tile([P, H, C], BF16)
        nc.gpsimd.dma_start(wproj[:], wproj_d.rearrange("(hb p) c -> p hb c", p=P))
        nc.scalar.dma_start(bf1c[:], bf1_d.rearrange("(t p) -> p t", p=P))

        # Load the exp table set immediately (PE is idle at t=0); every
        # later ACT op until the MLP (exp) uses this same set.
        nc.scalar.activation(
            out=dum[0:1, 0:1], in_=eps_t[0:1, 0:1],
            func=mybir.ActivationFunctionType.Exp, bias=0.0, scale=1.0,
        )

        h_fm = hfmp.tile([P, KT, N], BF16)
        h2_fm = h2p.tile([P, KT, N], BF16)
        o_fm = opool.tile([P, H, N], BF16)
        q_fm = qpool.tile([P, H, N], BF16)
        k_fm = kpool.tile([P, H, N], BF16)
        v_ext = vpool.tile([P, NT, H, VW + 1], BF16)
        nc.gpsimd.memset(v_ext[:, :, :, 0], 1.0)

        # ---------------- helpers ---------------------------------------
        def emit_ln_stats(nt, j, mus, vars_):
            """bn stats of x_tok[:, nt, :] -> mus[:, j], vars_[:, j]."""
            st = lnscr.tile([P, 2, nc.vector.BN_STATS_DIM], F32, tag="st")
            for i in range(2):
                nc.vector.bn_stats(
                    out=st[:, i, :], in_=x_tok[:, nt, i * 384:(i + 1) * 384]
                )
            mv = lnscr.tile([P, nc.vector.BN_AGGR_DIM], F32, tag="mv")
            nc.vector.bn_aggr(out=mv[:], in_=st[:])
            nc.vector.tensor_copy(mus[:, j:j + 1], mv[:, 0:1])
            nc.vector.tensor_copy(vars_[:, j:j + 1], mv[:, 1:2])

        def emit_newton(vars_, rstds, w, iters=3):
            """rstds[:, :w] = 1/sqrt(vars_[:, :w] + EPS) on DVE, batched.
            var is ~1 here (layernorm of ~unit-variance activations over
            768 dims), so a linear seed + 3 Newton steps converge to float
            accuracy."""
            vp = lnscr.tile([P, 4], F32, tag="vp")
            nc.vector.tensor_scalar_add(vp[:, :w], vars_[:, :w], EPS)
            nc.vector.tensor_scalar(
                rstds[:, :w], vp[:, :w], -0.5, 1.5, MULT, ADD
            )
            for _ in range(iters):
                t = lnscr.tile([P, 4], F32, tag="nt")
                nc.vector.tensor_mul(t[:, :w], rstds[:, :w], rstds[:, :w])
                nc.vector.tensor_mul(t[:, :w], t[:, :w], vp[:, :w])
                nc.vector.tensor_scalar(
                    t[:, :w], t[:, :w], -0.5, 1.5, MULT, ADD
                )
                nc.vector.tensor_mul(rstds[:, :w], rstds[:, :w], t[:, :w])

        def emit_badd(nt, brow):
            """x_tok[:, nt, :] += brow (after the LN that reads the
            pre-bias value, before the residual add that needs it)."""
            nc.vector.tensor_add(
                x_tok[:, nt, :], x_tok[:, nt, :], brow[:]
            )

        def emit_ln2(nt):
            """full per-tile LN2 (stats + per-tile Newton + DVE apply +
            transposes into h2_fm).  DVE apply: the ACT identity path is
            reserved for the attention-window exp stream."""
            mus = lnscr.tile([P, 1], F32, tag="mus2")
            vrs = lnscr.tile([P, 1], F32, tag="vrs2")
            rst = lnscr.tile([P, 1], F32, tag="rst2")
            emit_ln_stats(nt, 0, mus, vrs)
            emit_newton(vrs[:, 0:1], rst[:, 0:1], 1)
            nmu = lnscr.tile([P, 1], F32, tag="nmu2")
            nc.vector.tensor_scalar_mul(nmu[:], mus[:, 0:1], -1.0)
            h_t = lnscr.tile([P, C], BF16, tag="h")
            nc.vector.tensor_scalar(
                h_t[:], x_tok[:, nt, :], nmu[:], rst[:, 0:1], ADD, MULT
            )
            for kt in range(KT):
                tp = tpps.tile([P, P], BF16, tag="tp")
                nc.tensor.transpose(
                    tp[:], h_t[:, kt * P:(kt + 1) * P], ident[:]
                )
                nc.vector.tensor_copy(
                    h2_fm[:, kt, nt * P:(nt + 1) * P], tp[:]
                )
            emit_badd(nt, bf2b)

        def emit_qk1(h, which, nh):
            dst = q_fm if which == 0 else k_fm
            col0 = which * C + h * DH
            pq = work1.tile([P, NC_], F32, tag="w",
                            name=f"qk_{h}_{which}_{nh}")
            for kt in range(KT):
                nc.tensor.matmul(
                    pq[:DH, :],
                    wqkv[:, kt, col0:col0 + DH],
                    h_fm[:, kt, nh * NC_:(nh + 1) * NC_],
                    start=(kt == 0), stop=(kt == KT - 1),
                )
            nc.vector.tensor_scalar_add(
                dst[:DH, h, nh * NC_:(nh + 1) * NC_],
                pq[:DH, :],
                qkb[:DH, which * H + h:which * H + h + 1],
            )

        def emit_v(nt, half):
            pv = work1.tile([P, NC_], F32, tag="w", name=f"v_{nt}_{half}")
            c0 = 2 * C + half * 4 * DH
            for kt in range(KT):
                nc.tensor.matmul(
                    pv[:, 0:4 * DH],
                    h_fm[:, kt, nt * P:(nt + 1) * P],
                    wqkv[:, kt, c0:c0 + 4 * DH],
                    start=(kt == 0), stop=(kt == KT - 1),
                )
            nc.vector.tensor_copy(
                v_ext[:, nt, half * 4:(half + 1) * 4, 1:VW + 1],
                pv[:, 0:4 * DH].rearrange("p (h d) -> p h d", d=DH),
            )

        def emit_scores_begin(h, nh):
            return epool.tile([P, NT, NC_], BF16, tag="E", name=f"e_{h}_{nh}")

        def emit_scores_fill(e_t, h, nh, mt2):
            ps_s = sps.tile([P, 2, NC_], F32, tag="S",
                            name=f"s_{h}_{nh}_{mt2}")
            for sub in range(2):
                m0 = (2 * mt2 + sub) * P
                nc.tensor.matmul(
                    ps_s[:, sub, :],
                    k_fm[:DH, h, m0:m0 + P],
                    q_fm[:DH, h, nh * NC_:(nh + 1) * NC_],
                    start=True, stop=True,
                )
            nc.scalar.activation(
                out=e_t[:, 2 * mt2:2 * mt2 + 2, :], in_=ps_s[:],
                func=mybir.ActivationFunctionType.Exp,
                bias=0.0, scale=SCALE,
            )

        def emit_scores(h, nh):
            e_t = emit_scores_begin(h, nh)
            for mt2 in range(NT // 2):
                emit_scores_fill(e_t, h, nh, mt2)
            return e_t

        def emit_pv(h, nh, e_t):
            po = work1.tile([P, NC_], F32, tag="w", name=f"po_{h}_{nh}")
            for mt in range(NT):
                nc.tensor.matmul(
                    po[:VW + 1, :],
                    v_ext[:, mt, h, :],
                    e_t[:, mt, :],
                    start=(mt == 0), stop=(mt == NT - 1),
                )
            rs = rrow.tile([1, NC_], F32, tag="rs", name=f"rs_{h}_{nh}")
            nc.vector.reciprocal_approx_fast(out=rs[0:1, :], in_=po[0:1, :])
            rb = rrow.tile([P, NC_], F32, tag="rb", name=f"rb_{h}_{nh}")
            nc.gpsimd.partition_broadcast(rb[:VW + 1, :], rs[0:1, :])
            nc.vector.tensor_mul(
                o_fm[0:VW + 1, h, nh * NC_:(nh + 1) * NC_],
                po[0:VW + 1, :], rb[0:VW + 1, :],
            )

        def emit_proj_c(nt, c0, cw):
            pj = work1.tile([P, NC_], F32, tag="w", name=f"pj_{nt}_{c0}")
            for hb in range(H):
                nc.tensor.matmul(
                    pj[:, :cw],
                    o_fm[0:VW + 1, hb, nt * P:(nt + 1) * P],
                    wproj[0:VW + 1, hb, c0:c0 + cw],
                    start=(hb == 0), stop=(hb == H - 1),
                )
            nc.vector.tensor_add(
                x_tok[:, nt, c0:c0 + cw],
                pj[:, :cw], x_tok[:, nt, c0:c0 + cw],
            )

        def emit_proj(nt):
            emit_proj_c(nt, 0, 512)
            emit_proj_c(nt, 512, 256)

        _mlp_pools = []
        _mlp_weights = []

        def _alloc_mlp_weights():
            wqkvp.release()
            hfmp.release()
            wfc1p = tc.alloc_tile_pool(name="wfc1p", bufs=1, side="left")
            w1 = wfc1p.tile([P, KT, DFF], BF16, name="wfc1")
            w1r = wfc1_d.rearrange("(kt p) f -> p kt f", p=P)
            nc.sync.dma_start(w1[:, 0:3, :], w1r[:, 0:3, :])
            nc.gpsimd.dma_start(w1[:, 3:KT, :], w1r[:, 3:KT, :])
            _mlp_weights.append(w1)
            return [wfc1p]

        def _alloc_wfc2():
            # deferred past the q/k release so the SBUF high-water during
            # attention can afford a 4-deep e_t ring instead
            wfc2p = tc.alloc_tile_pool(name="wfc2p", bufs=1, side="left")
            w2 = wfc2p.tile([P, FFT, C], BF16, name="wfc2")
            w2r = wfc2_d.rearrange("(t p) c -> p t c", p=P)
            nc.sync.dma_start(w2[:, 0:12, :], w2r[:, 0:12, :])
            nc.gpsimd.dma_start(w2[:, 12:FFT, :], w2r[:, 12:FFT, :])
            _mlp_weights.append(w2)
            return [wfc2p]

        # ---------------- LN1 (batched rstd) + qkv ----------------------
        mus1 = lnscr.tile([P, NT], F32, tag="mus")
        vrs1 = lnscr.tile([P, NT], F32, tag="vrs")
        rst1 = lnscr.tile([P, NT], F32, tag="rst")

        def emit_ln_apply(nt, j, mus, rstds, dst_fm):
            """h = (x - mu) * rstd on ACT (identity: scale/bias, lives in
            every table set so no swap), then PE-transpose into dst_fm."""
            nmr = lnscr.tile([P, 1], F32, tag="nmr")
            nc.vector.scalar_tensor_tensor(
                nmr[:], mus[:, j:j + 1], -1.0, rstds[:, j:j + 1], MULT, MULT
            )
            h_t = lnscr.tile([P, C], BF16, tag="h")
            nc.scalar.activation(
                out=h_t[:], in_=x_tok[:, nt, :],
                func=mybir.ActivationFunctionType.Identity,
                bias=nmr[:], scale=rstds[:, j:j + 1],
            )
            for kt in range(KT):
                tp = tpps.tile([P, P], BF16, tag="tp")
                nc.tensor.transpose(
                    tp[:], h_t[:, kt * P:(kt + 1) * P], ident[:]
                )
                nc.vector.tensor_copy(
                    dst_fm[:, kt, nt * P:(nt + 1) * P], tp[:]
                )

        for nt in range(4):
            emit_ln_stats(nt, nt, mus1, vrs1)
        emit_newton(vrs1[:, 0:4], rst1[:, 0:4], 4, iters=1)
        for nt in range(4):
            emit_ln_apply(nt, nt, mus1, rst1, h_fm)
        # q over the first token half only needs LN1 of tiles 0-3;
        # LN1 of tiles 4-7 interleaves on DVE/ACT under these matmuls.
        for nt in range(4, NT):
            emit_ln_stats(nt, nt, mus1, vrs1)
        emit_newton(vrs1[:, 4:8], rst1[:, 4:8], 4, iters=1)
        for h in range(8):
            emit_qk1(h, 0, 0)
            if h < 4:
                emit_ln_apply(h + 4, h + 4, mus1, rst1, h_fm)
            else:
                emit_qk1(h - 4, 0, 1)
        # k (both halves); the first two score pairs trickle in here so
        # ACT gets a head start on the exp stream (its fills are spread to
        # avoid pacing PE at the exp rate).
        e0 = e1 = None
        for h in range(8):
            emit_qk1(h, 1, 0)
            emit_qk1(h, 1, 1)
            if h == 1:
                e0 = emit_scores_begin(0, 0)
                emit_scores_fill(e0, 0, 0, 0)
            elif h == 2:
                emit_scores_fill(e0, 0, 0, 1)
            elif h == 3:
                emit_scores_fill(e0, 0, 0, 2)
            elif h == 4:
                emit_scores_fill(e0, 0, 0, 3)
                e1 = emit_scores_begin(1, 0)
            elif h == 5:
                emit_scores_fill(e1, 1, 0, 0)
            elif h == 6:
                emit_scores_fill(e1, 1, 0, 1)
            elif h == 7:
                emit_scores_fill(e1, 1, 0, 2)
        e_live = {(0, 0): e0, (1, 0): e1}
        for nt in range(NT):
            emit_badd(nt, bpb)
        # all of v must precede the first PV (PV contracts over all of it)
        for nt in range(NT):
            emit_v(nt, 0)
            emit_v(nt, 1)
            if nt == 0:
                emit_scores_fill(e1, 1, 0, 3)

        # steady state: score pair i + PV of pair i-2, with PE filler
        # (remaining q-nh1 / proj / LN2) balancing the exp deficit.
        pairs = [(h, 0) for h in range(8)] + [(h, 1) for h in range(8)]
        for i in range(2, 16):
            e_live[pairs[i]] = emit_scores(*pairs[i])
            if 2 <= i < 6:
                emit_qk1(i + 2, 0, 1)
            if 10 <= i < 13:
                emit_proj_c(i - 9, 0, 512)
            emit_pv(*pairs[i - 2], e_live.pop(pairs[i - 2]))
            if i == 9:
                emit_proj_c(0, 0, 512)
                emit_proj_c(0, 512, 256)
            if 10 <= i < 13:
                emit_proj_c(i - 9, 512, 256)
            if i == 5:
                # wqkv/h_fm are fully consumed once V and all qk groups
                # are emitted: free the space and start the MLP loads.
                _mlp_pools.extend(_alloc_mlp_weights())
            if i == 13:
                emit_ln2(0)
            elif i == 14:
                emit_ln2(1)
            elif i == 15:
                emit_ln2(2)

        emit_pv(6, 1, e_live.pop((6, 1)))
        emit_ln2(3)
        emit_pv(7, 1, e_live.pop((7, 1)))

        if "q_fm" in tap_d:
            nc.sync.dma_start(
                tap_d["q_fm"].rearrange("(h p) n -> p h n", p=P), q_fm[:]
            )
        if "k_fm" in tap_d:
            nc.sync.dma_start(
                tap_d["k_fm"].rearrange("(h p) n -> p h n", p=P), k_fm[:]
            )
        if "o_fm" in tap_d:
            nc.sync.dma_start(
                tap_d["o_fm"].rearrange("(h p) n -> p h n", p=P), o_fm[:]
            )
        kpool.release()
        qpool.release()
        _mlp_pools.extend(_alloc_wfc2())
        epool.release()
        vpool.release()

        # ---------------- MLP -------------------------------------------
        wfc1, wfc2 = _mlp_weights
        gpool = tc.alloc_tile_pool(name="gpool", bufs=2, side="right")
        outs = tc.alloc_tile_pool(name="outs", bufs=2, side="right")

        def emit_fc1_chunk(g_t, half, ff0, ff1):
            for ff in range(ff0, ff1):
                pg = work1.tile([P, NC_], F32, tag="w", name=f"pg_{half}_{ff}")
                for kt in range(KT):
                    nc.tensor.matmul(
                        pg[:],
                        wfc1[:, kt, ff * P:(ff + 1) * P],
                        h2_fm[:, kt, half * NC_:(half + 1) * NC_],
                        start=(kt == 0), stop=(kt == KT - 1),
                    )
                nc.scalar.activation(
                    out=g_t[:, ff, :], in_=pg[:],
                    func=mybir.ActivationFunctionType.Gelu,
                    bias=bf1c[:, ff:ff + 1], scale=1.0,
                )

        def emit_fc2(q, g_t):
            qoff = (q % 2) * 256
            pa = [x2a.tile([P, 512], F32, tag="a", name=f"pa{q}_{j}")
                  for j in range(2)]
            pb = [x2b.tile([P, 256], F32, tag="b", name=f"pb{q}_{j}")
                  for j in range(2)]
            for ff in range(FFT):
                for j in range(2):
                    lhsT = g_t[:, ff, qoff + j * P:qoff + (j + 1) * P]
                    nc.tensor.matmul(
                        pa[j][:], lhsT, wfc2[:, ff, 0:512],
                        start=(ff == 0), stop=(ff == FFT - 1),
                    )
                    nc.tensor.matmul(
                        pb[j][:], lhsT, wfc2[:, ff, 512:768],
                        start=(ff == 0), stop=(ff == FFT - 1),
                    )
            for j in range(2):
                nt = 2 * q + j
                o_t = outs.tile([P, C], F32, tag="y", name=f"y_{q}_{j}")
                nc.vector.tensor_add(
                    o_t[:, 0:512], pa[j][:], x_tok[:, nt, 0:512]
                )
                nc.vector.tensor_add(
                    o_t[:, 512:768], pb[j][:], x_tok[:, nt, 512:768]
                )
                nc.sync.dma_start(y_d[nt * P:(nt + 1) * P, :], o_t[:])

        g0 = gpool.tile([P, FFT, NC_], BF16, tag="g", name="g_0")
        emit_fc1_chunk(g0, 0, 0, 6)
        emit_proj(4)
        emit_fc1_chunk(g0, 0, 6, 12)
        emit_proj(5)
        emit_fc1_chunk(g0, 0, 12, 18)
        emit_proj(6)
        emit_fc1_chunk(g0, 0, 18, 24)
        emit_proj(7)
        for nt in range(4, 8):
            emit_ln2(nt)

        if "x1" in tap_d:  # note: includes +b_fc2 (folded early)
            nc.sync.dma_start(
                tap_d["x1"].rearrange("(nt p) c -> p nt c", p=P), x_tok[:]
            )
        if "h2_fm" in tap_d:
            nc.sync.dma_start(
                tap_d["h2_fm"].rearrange("(kt p) n -> p kt n", p=P), h2_fm[:]
            )

        sps.release()
        tpps.release()
        x2a = tc.alloc_tile_pool(name="x2a", bufs=4, space="PSUM")
        x2b = tc.alloc_tile_pool(name="x2b", bufs=2, space="PSUM")
        emit_fc2(0, g0)
        emit_fc2(1, g0)
        g1 = gpool.tile([P, FFT, NC_], BF16, tag="g", name="g_1")
        emit_fc1_chunk(g1, 1, 0, 24)
        emit_fc2(2, g1)
        emit_fc2(3, g1)

        x2b.release()
        x2a.release()
        work1.release()
        outs.release()
        gpool.release()
        for pool in reversed(_mlp_pools):
            pool.release()
        rrow.release()
        opool.release()
        lnscr.release()
        h2p.release()
        wprojp.release()
        xpool.release()
        consts.release()

    nc.compile()
    return nc


def _prep_inputs(inputs):
    """Host-side prep (exact refactoring of LN gains/biases into weights)."""
    f = lambda k: np.asarray(inputs[k], dtype=np.float32)
    x = f("x")
    w_qkv, w_proj, w_fc1, w_fc2 = f("w_qkv"), f("w_proj"), f("w_fc1"), f("w_fc2")
    ln1_g, ln1_b, ln2_g, ln2_b = f("ln1_g"), f("ln1_b"), f("ln2_g"), f("ln2_b")
    b_proj, b_fc1, b_fc2 = f("b_proj"), f("b_fc1"), f("b_fc2")

    bf = ml_dtypes.bfloat16
    w_qkv_e = ln1_g[:, None] * w_qkv
    qkv_bias = ln1_b @ w_qkv  # [2304]
    qk_bias = np.zeros((P, 2 * H), dtype=np.float32)
    for which in range(2):
        for h in range(H):
            qk_bias[0:DH, which * H + h] = qkv_bias[
                which * C + h * DH: which * C + (h + 1) * DH
            ]
    vb = qkv_bias[2 * C: 3 * C]  # v bias passes through softmax additively
    b_proj_e = b_proj + vb @ w_proj
    # head-aligned w_proj rows: block h rows 1..96 (row 0 pairs with colsum row)
    w_proj_p = np.zeros((H * P, C), dtype=np.float32)
    for h in range(H):
        w_proj_p[h * P + 1: h * P + 1 + DH, :] = w_proj[h * DH:(h + 1) * DH, :]
    w_fc1_e = ln2_g[:, None] * w_fc1
    b_fc1_e = b_fc1 + ln2_b @ w_fc1

    common = {
        "w_qkv_e": np.ascontiguousarray(w_qkv_e.astype(bf)),
        "w_proj_p": np.ascontiguousarray(w_proj_p.astype(bf)),
        "w_fc1_e": np.ascontiguousarray(w_fc1_e.astype(bf)),
        "w_fc2": np.ascontiguousarray(w_fc2.astype(bf)),
        "qk_bias": qk_bias,
        "b_proj_e": np.ascontiguousarray(b_proj_e.astype(bf)),
        "b_fc1_e": b_fc1_e,
        "b_fc2": np.ascontiguousarray(b_fc2.astype(bf)),
    }
    xb = x.astype(bf)
    return [dict(common, x_bf=np.ascontiguousarray(xb[i])) for i in range(8)]


def kernel(**inputs):
    if "nc" not in _CACHED:
        _CACHED["nc"] = build()
    nc = _CACHED["nc"]
    in_maps = _prep_inputs(inputs)
    res = run_bass_kernel_spmd(nc, in_maps, core_ids=list(range(8)))
    out = np.stack([res.results[i]["y"] for i in range(8)], axis=0)
    return out.astype(np.float32)



# revision 4
# speedup vs baseline: 1.1159x; 1.1159x over previous
"""Transformer block (pre-norm attn + MLP) on 8 NeuronCores, data-parallel
over batch. Full inputs in, full outputs out; each core runs one batch
element x[i] : [1024, 768] through an identical Bass/Tile kernel.

v3: attention path in fp8 (e4m3) with DoubleRow matmuls.

Host-side exact refactoring (as v2) plus fp8 quantization of the
attention weights:
  - LN gains fold into the following matmul weights: diag(g) @ W.
  - LN biases fold into: per-column bias on q/k (x16), b_proj_eff,
    b_fc1_eff.
  - w_qkv, w_proj quantized to e4m3 at scale 512 (max |w|*512 ~ 100).
  - w_proj rows re-laid-out head-aligned: block h rows 1..96 (row 0 pairs
    with the attention colsum row; zero).
  - x cast to bf16; MLP weights bf16 (fp8 MLP breaches the 2e-2 gate).

fp8 scaling scheme (S_A = 16 activations, S_W = 512 weights):
  h_fm8 = 16*ln1(x) -> q8/k8 = 16*q (psum/512 + 16*bias), v8 = 16*v.
  scores psum = 256*(q.k); exp arg = psum*SCALE/256 - 4 (the -4 keeps
  e below e4m3 max 240; softmax is shift-invariant incl. the colsum row).
  e8 = exp(s-4); PV row0 = colsum; o8 = 16*o after reciprocal bcast.
  proj psum = 16*512*(o@wp) -> residual add with 1/8192 fold (stt).

DoubleRow (contraction 256/matmul, 2x PE throughput) on qkv, v, PV (m-tile
pairs), proj (head-block pairs). Scores stay at 96-contraction (fp8 at
bf16 rate) in this version.

Device dataflow per core: as v2 (single fully-pipelined emission).
"""
import numpy as np
import ml_dtypes

import concourse.bass as bass
from concourse import bacc, mybir
from concourse.bass_utils import run_bass_kernel_spmd
from concourse.masks import make_identity
from concourse.tile import TileContext

P = 128
N = 1024          # tokens per core (batch element)
C = 768           # model dim
H = 8             # heads
DH = C // H       # 96
DFF = 4 * C       # 3072
NT = N // P       # 8 token tiles
KT = C // P       # 6 feature tiles
FFT = DFF // P    # 24 ff tiles
NH = 2            # halves of the token axis for attention
NC_ = N // NH     # 512
EPS = 1e-5
SCALE = DH ** -0.5
VW = DH           # per-head v width (plus a leading ones column)
VWP = 104         # padded per-head v stride: 8*104 % 16 == 0 (DoubleRow
                  # pair-axis step must be a 16B multiple)
S_A = 16.0        # fp8 activation scale
S_W = 512.0       # fp8 weight scale
ESHIFT = -4.0     # exp downshift so e stays < 240

F32 = mybir.dt.float32
BF16 = mybir.dt.bfloat16
F8 = mybir.dt.float8e4
MULT = mybir.AluOpType.mult
ADD = mybir.AluOpType.add
DR = mybir.MatmulPerfMode.DoubleRow

_CACHED = {}


def build(taps=()):
    nc = bacc.Bacc("TRN2", debug=False)

    x_d = nc.dram_tensor("x_bf", [N, C], BF16, kind="ExternalInput")
    wqkv_d = nc.dram_tensor("w_qkv_e", [C, 3 * C], F8, kind="ExternalInput")
    wproj_d = nc.dram_tensor("w_proj_p", [H * P, C], F8, kind="ExternalInput")
    wfc1_d = nc.dram_tensor("w_fc1_e", [C, DFF], BF16, kind="ExternalInput")
    wfc2_d = nc.dram_tensor("w_fc2", [DFF, C], BF16, kind="ExternalInput")
    qkb_d = nc.dram_tensor("qk_bias", [P, 2 * H], F32, kind="ExternalInput")
    bp_d = nc.dram_tensor("b_proj_e", [C], BF16, kind="ExternalInput")
    bf1_d = nc.dram_tensor("b_fc1_e", [DFF], F32, kind="ExternalInput")
    bf2_d = nc.dram_tensor("b_fc2", [C], BF16, kind="ExternalInput")
    y_d = nc.dram_tensor("y", [N, C], F32, kind="ExternalOutput")

    tap_d = {}
    for name, shape, dt in [
        ("h_fm", [C, N], F8),
        ("q_fm", [H * P, N], F8),
        ("k_fm", [H * P, N], F8),
        ("o_fm", [H * P, N], F8),
        ("x1", [N, C], BF16),
        ("h2_fm", [C, N], BF16),
    ]:
        if name in taps:
            tap_d[name] = nc.dram_tensor(
                "tap_" + name, shape, dt, kind="ExternalOutput"
            )

    with TileContext(nc) as tc:
        # ---------------- SBUF pools, LEFT stack (bottom -> top) --------
        consts = tc.alloc_tile_pool(name="consts", bufs=1, side="left")
        xpool = tc.alloc_tile_pool(name="xpool", bufs=1, side="left")
        wprojp = tc.alloc_tile_pool(name="wprojp", bufs=1, side="left")
        h2p = tc.alloc_tile_pool(name="h2p", bufs=1, side="left")
        lnscr = tc.alloc_tile_pool(name="lnscr", bufs=2, side="left")
        hfmp = tc.alloc_tile_pool(name="hfmp", bufs=1, side="left")
        wqkvp = tc.alloc_tile_pool(name="wqkvp", bufs=1, side="left")

        # ---------------- SBUF pools, RIGHT stack -----------------------
        opool = tc.alloc_tile_pool(name="opool", bufs=1, side="right")
        rrow = tc.alloc_tile_pool(name="rrow", bufs=1, side="right")
        vpool = tc.alloc_tile_pool(name="vpool", bufs=1, side="right")
        epool = tc.alloc_tile_pool(name="epool", bufs=4, side="right")
        qpool = tc.alloc_tile_pool(name="qpool", bufs=1, side="right")
        kpool = tc.alloc_tile_pool(name="kpool", bufs=1, side="right")

        # ---------------- PSUM pools ------------------------------------
        work1 = tc.alloc_tile_pool(name="work1", bufs=2, space="PSUM")
        tpps = tc.alloc_tile_pool(name="tpps", bufs=2, space="PSUM")
        sps = tc.alloc_tile_pool(name="sps", bufs=2, space="PSUM")

        # ---------------- constants ------------------------------------
        ident = consts.tile([P, P], BF16)
        make_identity(nc, ident)
        eps_t = consts.tile([P, 1], F32)
        nc.vector.memset(eps_t, EPS)
        esh_t = consts.tile([P, 1], F32)
        nc.vector.memset(esh_t, ESHIFT)
        dum = consts.tile([1, 1], F32)
        qkb = consts.tile([P, 2 * H], F32)
        bf1c = consts.tile([P, FFT], F32)
        bpb = consts.tile([P, C], BF16)
        bf2b = consts.tile([P, C], BF16)

        # ---------------- big tiles + DMAs ------------------------------
        # spread across engine queues: each queue moves ~130 GB/s, so the
        # startup loads (x then wq/wk) go wide, ordered by first consumer
        x_tok = xpool.tile([P, NT, C], BF16)
        xr = x_d.rearrange("(nt p) c -> p nt c", p=P)
        wqkv = wqkvp.tile([P, KT, 3 * C], F8)
        wr = wqkv_d.rearrange("(kt p) o -> p kt o", p=P)
        nc.sync.dma_start(x_tok[:, 0:2, :], xr[:, 0:2, :])
        nc.gpsimd.dma_start(x_tok[:, 2:4, :], xr[:, 2:4, :])
        nc.scalar.dma_start(x_tok[:, 4:NT, :], xr[:, 4:NT, :])
        nc.sync.dma_start(wqkv[:, :, 0:384], wr[:, :, 0:384])
        nc.gpsimd.dma_start(wqkv[:, :, 384:C], wr[:, :, 384:C])
        nc.scalar.dma_start(qkb[:], qkb_d[:, :])
        nc.sync.dma_start(wqkv[:, :, C:C + 384], wr[:, :, C:C + 384])
        nc.gpsimd.dma_start(wqkv[:, :, C + 384:2 * C], wr[:, :, C + 384:2 * C])
        nc.sync.dma_start(wqkv[:, :, 2 * C:2 * C + 384], wr[:, :, 2 * C:2 * C + 384])
        nc.gpsimd.dma_start(wqkv[:, :, 2 * C + 384:3 * C], wr[:, :, 2 * C + 384:])

        brow1 = consts.tile([1, C], BF16)
        brow2 = consts.tile([1, C], BF16)
        nc.scalar.dma_start(
            brow1[0:1, :], bass.AP(tensor=bp_d, offset=0, ap=[[0, 1], [1, C]])
        )
        nc.scalar.dma_start(
            brow2[0:1, :], bass.AP(tensor=bf2_d, offset=0, ap=[[0, 1], [1, C]])
        )
        nc.gpsimd.partition_broadcast(bpb[:, :], brow1[0:1, :])
        nc.gpsimd.partition_broadcast(bf2b[:, :], brow2[0:1, :])

        wproj = wprojp.tile([P, H, C], F8)
        nc.gpsimd.dma_start(wproj[:], wproj_d.rearrange("(hb p) c -> p hb c", p=P))
        nc.scalar.dma_start(bf1c[:], bf1_d.rearrange("(t p) -> p t", p=P))

        # Load the exp table set immediately (PE is idle at t=0); every
        # later ACT op until the MLP (exp) uses this same set.
        nc.scalar.activation(
            out=dum[0:1, 0:1], in_=eps_t[0:1, 0:1],
            func=mybir.ActivationFunctionType.Exp, bias=0.0, scale=1.0,
        )

        h_fm = hfmp.tile([P, KT, N], F8)
        h2_fm = h2p.tile([P, KT, N], BF16)
        o_fm = opool.tile([P, H, N], F8)
        q_fm = qpool.tile([P, H, N], F8)
        k_fm = kpool.tile([P, H, N], F8)
        v_ext = vpool.tile([P, NT, H, VWP], F8)
        nc.gpsimd.memset(v_ext[:, :, :, 0], 1.0)

        # ---------------- helpers ---------------------------------------
        def emit_ln_stats(nt, j, mus, vars_):
            """bn stats of x_tok[:, nt, :] -> mus[:, j], vars_[:, j]."""
            st = lnscr.tile([P, 2, nc.vector.BN_STATS_DIM], F32, tag="st")
            for i in range(2):
                nc.vector.bn_stats(
                    out=st[:, i, :], in_=x_tok[:, nt, i * 384:(i + 1) * 384]
                )
            mv = lnscr.tile([P, nc.vector.BN_AGGR_DIM], F32, tag="mv")
            nc.vector.bn_aggr(out=mv[:], in_=st[:])
            nc.vector.tensor_copy(mus[:, j:j + 1], mv[:, 0:1])
            nc.vector.tensor_copy(vars_[:, j:j + 1], mv[:, 1:2])

        def emit_newton(vars_, rstds, w, iters=3, final_scale=None):
            """rstds[:, :w] = 1/sqrt(vars_[:, :w] + EPS) on DVE, batched.
            var is ~1 here (layernorm of ~unit-variance activations over
            768 dims), so a linear seed + 3 Newton steps converge to float
            accuracy.  final_scale folds the fp8 activation scale in."""
            vp = lnscr.tile([P, 4], F32, tag="vp")
            nc.vector.tensor_scalar_add(vp[:, :w], vars_[:, :w], EPS)
            nc.vector.tensor_scalar(
                rstds[:, :w], vp[:, :w], -0.5, 1.5, MULT, ADD
            )
            for it in range(iters):
                t = lnscr.tile([P, 4], F32, tag="nt")
                nc.vector.tensor_mul(t[:, :w], rstds[:, :w], rstds[:, :w])
                nc.vector.tensor_mul(t[:, :w], t[:, :w], vp[:, :w])
                nc.vector.tensor_scalar(
                    t[:, :w], t[:, :w], -0.5, 1.5, MULT, ADD
                )
                if final_scale is not None and it == iters - 1:
                    nc.vector.tensor_scalar_mul(t[:, :w], t[:, :w], final_scale)
                nc.vector.tensor_mul(rstds[:, :w], rstds[:, :w], t[:, :w])

        def emit_badd(nt, brow):
            """x_tok[:, nt, :] += brow (after the LN that reads the
            pre-bias value, before the residual add that needs it)."""
            nc.vector.tensor_add(
                x_tok[:, nt, :], x_tok[:, nt, :], brow[:]
            )

        def emit_ln2(nt):
            """full per-tile LN2 (stats + per-tile Newton + DVE apply +
            transposes into h2_fm).  DVE apply: the ACT identity path is
            reserved for the attention-window exp stream."""
            mus = lnscr.tile([P, 1], F32, tag="mus2")
            vrs = lnscr.tile([P, 1], F32, tag="vrs2")
            rst = lnscr.tile([P, 1], F32, tag="rst2")
            emit_ln_stats(nt, 0, mus, vrs)
            emit_newton(vrs[:, 0:1], rst[:, 0:1], 1)
            nmu = lnscr.tile([P, 1], F32, tag="nmu2")
            nc.vector.tensor_scalar_mul(nmu[:], mus[:, 0:1], -1.0)
            h_t = lnscr.tile([P, C], BF16, tag="h")
            nc.vector.tensor_scalar(
                h_t[:], x_tok[:, nt, :], nmu[:], rst[:, 0:1], ADD, MULT
            )
            for kt in range(KT):
                tp = tpps.tile([P, P], BF16, tag="tp")
                nc.tensor.transpose(
                    tp[:], h_t[:, kt * P:(kt + 1) * P], ident[:]
                )
                nc.vector.tensor_copy(
                    h2_fm[:, kt, nt * P:(nt + 1) * P], tp[:]
                )
            emit_badd(nt, bf2b)

        def emit_qk1(h, which, nh):
            """q or k for one head/half: DoubleRow fp8, psum/512 + 16*bias
            -> fp8 [96, 512] slice of q_fm/k_fm."""
            dst = q_fm if which == 0 else k_fm
            col0 = which * C + h * DH
            pq = work1.tile([P, NC_], F32, tag="w",
                            name=f"qk_{h}_{which}_{nh}")
            for kp in range(KT // 2):
                nc.tensor.matmul(
                    pq[:DH, :],
                    wqkv[:, 2 * kp:2 * kp + 2, col0:col0 + DH],
                    h_fm[:, 2 * kp:2 * kp + 2, nh * NC_:(nh + 1) * NC_],
                    start=(kp == 0), stop=(kp == KT // 2 - 1),
                    perf_mode=DR,
                )
            nc.vector.tensor_scalar(
                dst[:DH, h, nh * NC_:(nh + 1) * NC_],
                pq[:DH, :],
                1.0 / S_W,
                qkb[:DH, which * H + h:which * H + h + 1],
                MULT, ADD,
            )

        def emit_v(nt, half):
            pv = work1.tile([P, NC_], F32, tag="w", name=f"v_{nt}_{half}")
            c0 = 2 * C + half * 4 * DH
            for kp in range(KT // 2):
                nc.tensor.matmul(
                    pv[:, 0:4 * DH],
                    h_fm[:, 2 * kp:2 * kp + 2, nt * P:(nt + 1) * P],
                    wqkv[:, 2 * kp:2 * kp + 2, c0:c0 + 4 * DH],
                    start=(kp == 0), stop=(kp == KT // 2 - 1),
                    perf_mode=DR,
                )
            nc.vector.tensor_scalar_mul(
                v_ext[:, nt, half * 4:(half + 1) * 4, 1:VW + 1],
                pv[:, 0:4 * DH].rearrange("p (h d) -> p h d", d=DH),
                1.0 / S_W,
            )

        def emit_scores_begin(h, nh):
            return epool.tile([P, NT, NC_], F8, tag="E", name=f"e_{h}_{nh}")

        def emit_scores_fill(e_t, h, nh, mt2):
            ps_s = sps.tile([P, 2, NC_], F32, tag="S",
                            name=f"s_{h}_{nh}_{mt2}")
            for sub in range(2):
                m0 = (2 * mt2 + sub) * P
                nc.tensor.matmul(
                    ps_s[:, sub, :],
                    k_fm[:DH, h, m0:m0 + P],
                    q_fm[:DH, h, nh * NC_:(nh + 1) * NC_],
                    start=True, stop=True,
                )
            nc.scalar.activation(
                out=e_t[:, 2 * mt2:2 * mt2 + 2, :], in_=ps_s[:],
                func=mybir.ActivationFunctionType.Exp,
                bias=esh_t[:], scale=SCALE / (S_A * S_A),
            )

        def emit_scores(h, nh):
            e_t = emit_scores_begin(h, nh)
            for mt2 in range(NT // 2):
                emit_scores_fill(e_t, h, nh, mt2)
            return e_t

        def emit_pv(h, nh, e_t):
            po = work1.tile([P, NC_], F32, tag="w", name=f"po_{h}_{nh}")
            for mp in range(NT // 2):
                nc.tensor.matmul(
                    po[:VW + 1, :],
                    v_ext[:, 2 * mp:2 * mp + 2, h, 0:VW + 1],
                    e_t[:, 2 * mp:2 * mp + 2, :],
                    start=(mp == 0), stop=(mp == NT // 2 - 1),
                    perf_mode=DR,
                )
            rs = rrow.tile([1, NC_], F32, tag="rs", name=f"rs_{h}_{nh}")
            nc.vector.reciprocal_approx_fast(out=rs[0:1, :], in_=po[0:1, :])
            rb = rrow.tile([P, NC_], F32, tag="rb", name=f"rb_{h}_{nh}")
            nc.gpsimd.partition_broadcast(rb[:VW + 1, :], rs[0:1, :])
            nc.vector.tensor_mul(
                o_fm[0:VW + 1, h, nh * NC_:(nh + 1) * NC_],
                po[0:VW + 1, :], rb[0:VW + 1, :],
            )

        def emit_proj_c(nt, c0, cw):
            pj = work1.tile([P, NC_], F32, tag="w", name=f"pj_{nt}_{c0}")
            for hp in range(H // 2):
                nc.tensor.matmul(
                    pj[:, :cw],
                    o_fm[0:VW + 1, 2 * hp:2 * hp + 2, nt * P:(nt + 1) * P],
                    wproj[0:VW + 1, 2 * hp:2 * hp + 2, c0:c0 + cw],
                    start=(hp == 0), stop=(hp == H // 2 - 1),
                    perf_mode=DR,
                )
            nc.vector.scalar_tensor_tensor(
                x_tok[:, nt, c0:c0 + cw],
                pj[:, :cw], 1.0 / (S_A * S_W), x_tok[:, nt, c0:c0 + cw],
                MULT, ADD,
            )

        def emit_proj(nt):
            emit_proj_c(nt, 0, 512)
            emit_proj_c(nt, 512, 256)

        _mlp_pools = []
        _mlp_weights = []

        def _alloc_mlp_weights():
            wqkvp.release()
            hfmp.release()
            wfc1p = tc.alloc_tile_pool(name="wfc1p", bufs=1, side="left")
            w1 = wfc1p.tile([P, KT, DFF], BF16, name="wfc1")
            w1r = wfc1_d.rearrange("(kt p) f -> p kt f", p=P)
            nc.sync.dma_start(w1[:, 0:3, :], w1r[:, 0:3, :])
            nc.gpsimd.dma_start(w1[:, 3:KT, :], w1r[:, 3:KT, :])
            _mlp_weights.append(w1)
            return [wfc1p]

        def _alloc_wfc2():
            # deferred past the q/k release so the SBUF high-water during
            # attention can afford a 4-deep e_t ring instead
            wfc2p = tc.alloc_tile_pool(name="wfc2p", bufs=1, side="left")
            w2 = wfc2p.tile([P, FFT, C], BF16, name="wfc2")
            w2r = wfc2_d.rearrange("(t p) c -> p t c", p=P)
            nc.sync.dma_start(w2[:, 0:12, :], w2r[:, 0:12, :])
            nc.gpsimd.dma_start(w2[:, 12:FFT, :], w2r[:, 12:FFT, :])
            _mlp_weights.append(w2)
            return [wfc2p]

        # ---------------- LN1 (batched rstd) + qkv ----------------------
        mus1 = lnscr.tile([P, NT], F32, tag="mus")
        vrs1 = lnscr.tile([P, NT], F32, tag="vrs")
        rst1 = lnscr.tile([P, NT], F32, tag="rst")

        def emit_ln_apply(nt, j, mus, rstds, dst_fm):
            """h = (x - mu) * rstd * 16 on ACT (identity: scale/bias, lives
            in every table set so no swap), then PE-transpose into dst_fm
            (fp8, scale 16)."""
            nmr = lnscr.tile([P, 1], F32, tag="nmr")
            nc.vector.scalar_tensor_tensor(
                nmr[:], mus[:, j:j + 1], -1.0, rstds[:, j:j + 1], MULT, MULT
            )
            h_t = lnscr.tile([P, C], BF16, tag="h")
            nc.scalar.activation(
                out=h_t[:], in_=x_tok[:, nt, :],
                func=mybir.ActivationFunctionType.Identity,
                bias=nmr[:], scale=rstds[:, j:j + 1],
            )
            for kt in range(KT):
                tp = tpps.tile([P, P], BF16, tag="tp")
                nc.tensor.transpose(
                    tp[:], h_t[:, kt * P:(kt + 1) * P], ident[:]
                )
                nc.vector.tensor_copy(
                    dst_fm[:, kt, nt * P:(nt + 1) * P], tp[:]
                )

        for nt in range(4):
            emit_ln_stats(nt, nt, mus1, vrs1)
        emit_newton(vrs1[:, 0:4], rst1[:, 0:4], 4, iters=1, final_scale=S_A)
        for nt in range(4):
            emit_ln_apply(nt, nt, mus1, rst1, h_fm)
        # q over the first token half only needs LN1 of tiles 0-3;
        # LN1 of tiles 4-7 interleaves on DVE/ACT under these matmuls.
        for nt in range(4, NT):
            emit_ln_stats(nt, nt, mus1, vrs1)
        emit_newton(vrs1[:, 4:8], rst1[:, 4:8], 4, iters=1, final_scale=S_A)
        for h in range(8):
            emit_qk1(h, 0, 0)
            if h < 4:
                emit_ln_apply(h + 4, h + 4, mus1, rst1, h_fm)
            else:
                emit_qk1(h - 4, 0, 1)
        # k (both halves); the first two score pairs trickle in here so
        # ACT gets a head start on the exp stream (its fills are spread to
        # avoid pacing PE at the exp rate).
        e0 = e1 = None
        for h in range(8):
            emit_qk1(h, 1, 0)
            emit_qk1(h, 1, 1)
            if h == 1:
                e0 = emit_scores_begin(0, 0)
                emit_scores_fill(e0, 0, 0, 0)
            elif h == 2:
                emit_scores_fill(e0, 0, 0, 1)
            elif h == 3:
                emit_scores_fill(e0, 0, 0, 2)
            elif h == 4:
                emit_scores_fill(e0, 0, 0, 3)
                e1 = emit_scores_begin(1, 0)
            elif h == 5:
                emit_scores_fill(e1, 1, 0, 0)
            elif h == 6:
                emit_scores_fill(e1, 1, 0, 1)
            elif h == 7:
                emit_scores_fill(e1, 1, 0, 2)
        e_live = {(0, 0): e0, (1, 0): e1}
        for nt in range(NT):
            emit_badd(nt, bpb)
        # all of v must precede the first PV (PV contracts over all of it)
        for nt in range(NT):
            emit_v(nt, 0)
            emit_v(nt, 1)
            if nt == 0:
                emit_scores_fill(e1, 1, 0, 3)

        # steady state: score pair i + PV of pair i-2, with PE filler
        # (remaining q-nh1 / proj / LN2) balancing the exp deficit.
        pairs = [(h, 0) for h in range(8)] + [(h, 1) for h in range(8)]
        for i in range(2, 16):
            e_live[pairs[i]] = emit_scores(*pairs[i])
            if 2 <= i < 6:
                emit_qk1(i + 2, 0, 1)
            if 10 <= i < 13:
                emit_proj_c(i - 9, 0, 512)
            emit_pv(*pairs[i - 2], e_live.pop(pairs[i - 2]))
            if i == 9:
                emit_proj_c(0, 0, 512)
                emit_proj_c(0, 512, 256)
            if 10 <= i < 13:
                emit_proj_c(i - 9, 512, 256)
            if i == 5:
                # wqkv/h_fm are fully consumed once V and all qk groups
                # are emitted: free the space and start the MLP loads.
                _mlp_pools.extend(_alloc_mlp_weights())
            if i == 13:
                emit_ln2(0)
            elif i == 14:
                emit_ln2(1)
            elif i == 15:
                emit_ln2(2)

        emit_pv(6, 1, e_live.pop((6, 1)))
        emit_ln2(3)
        emit_pv(7, 1, e_live.pop((7, 1)))

        if "q_fm" in tap_d:
            nc.sync.dma_start(
                tap_d["q_fm"].rearrange("(h p) n -> p h n", p=P), q_fm[:]
            )
        if "k_fm" in tap_d:
            nc.sync.dma_start(
                tap_d["k_fm"].rearrange("(h p) n -> p h n", p=P), k_fm[:]
            )
        if "o_fm" in tap_d:
            nc.sync.dma_start(
                tap_d["o_fm"].rearrange("(h p) n -> p h n", p=P), o_fm[:]
            )
        kpool.release()
        qpool.release()
        _mlp_pools.extend(_alloc_wfc2())
        epool.release()
        vpool.release()

        # ---------------- MLP -------------------------------------------
        wfc1, wfc2 = _mlp_weights
        gpool = tc.alloc_tile_pool(name="gpool", bufs=2, side="right")
        outs = tc.alloc_tile_pool(name="outs", bufs=2, side="right")

        def emit_fc1_chunk(g_t, half, ff0, ff1):
            for ff in range(ff0, ff1):
                pg = work1.tile([P, NC_], F32, tag="w", name=f"pg_{half}_{ff}")
                for kt in range(KT):
                    nc.tensor.matmul(
                        pg[:],
                        wfc1[:, kt, ff * P:(ff + 1) * P],
                        h2_fm[:, kt, half * NC_:(half + 1) * NC_],
                        start=(kt == 0), stop=(kt == KT - 1),
                    )
                nc.scalar.activation(
                    out=g_t[:, ff, :], in_=pg[:],
                    func=mybir.ActivationFunctionType.Gelu,
                    bias=bf1c[:, ff:ff + 1], scale=1.0,
                )

        def emit_fc2(q, g_t):
            qoff = (q % 2) * 256
            pa = [x2a.tile([P, 512], F32, tag="a", name=f"pa{q}_{j}")
                  for j in range(2)]
            pb = [x2b.tile([P, 256], F32, tag="b", name=f"pb{q}_{j}")
                  for j in range(2)]
            for ff in range(FFT):
                for j in range(2):
                    lhsT = g_t[:, ff, qoff + j * P:qoff + (j + 1) * P]
                    nc.tensor.matmul(
                        pa[j][:], lhsT, wfc2[:, ff, 0:512],
                        start=(ff == 0), stop=(ff == FFT - 1),
                    )
                    nc.tensor.matmul(
                        pb[j][:], lhsT, wfc2[:, ff, 512:768],
                        start=(ff == 0), stop=(ff == FFT - 1),
                    )
            for j in range(2):
                nt = 2 * q + j
                o_t = outs.tile([P, C], F32, tag="y", name=f"y_{q}_{j}")
                nc.vector.tensor_add(
                    o_t[:, 0:512], pa[j][:], x_tok[:, nt, 0:512]
                )
                nc.vector.tensor_add(
                    o_t[:, 512:768], pb[j][:], x_tok[:, nt, 512:768]
                )
                nc.sync.dma_start(y_d[nt * P:(nt + 1) * P, :], o_t[:])

        g0 = gpool.tile([P, FFT, NC_], BF16, tag="g", name="g_0")
        emit_fc1_chunk(g0, 0, 0, 6)
        emit_proj(4)
        emit_fc1_chunk(g0, 0, 6, 12)
        emit_proj(5)
        emit_fc1_chunk(g0, 0, 12, 18)
        emit_proj(6)
        emit_fc1_chunk(g0, 0, 18, 24)
        emit_proj(7)
        for nt in range(4, 8):
            emit_ln2(nt)

        if "x1" in tap_d:  # note: includes +b_fc2 (folded early)
            nc.sync.dma_start(
                tap_d["x1"].rearrange("(nt p) c -> p nt c", p=P), x_tok[:]
            )
        if "h2_fm" in tap_d:
            nc.sync.dma_start(
                tap_d["h2_fm"].rearrange("(kt p) n -> p kt n", p=P), h2_fm[:]
            )

        sps.release()
        tpps.release()
        x2a = tc.alloc_tile_pool(name="x2a", bufs=4, space="PSUM")
        x2b = tc.alloc_tile_pool(name="x2b", bufs=2, space="PSUM")
        emit_fc2(0, g0)
        emit_fc2(1, g0)
        g1 = gpool.tile([P, FFT, NC_], BF16, tag="g", name="g_1")
        emit_fc1_chunk(g1, 1, 0, 24)
        emit_fc2(2, g1)
        emit_fc2(3, g1)

        x2b.release()
        x2a.release()
        work1.release()
        outs.release()
        gpool.release()
        for pool in reversed(_mlp_pools):
            pool.release()
        rrow.release()
        opool.release()
        lnscr.release()
        h2p.release()
        wprojp.release()
        xpool.release()
        consts.release()

    nc.compile()
    return nc


def _prep_inputs(inputs):
    """Host-side prep (exact refactoring of LN gains/biases into weights,
    fp8 quantization of the attention weights)."""
    f = lambda k: np.asarray(inputs[k], dtype=np.float32)
    x = f("x")
    w_qkv, w_proj, w_fc1, w_fc2 = f("w_qkv"), f("w_proj"), f("w_fc1"), f("w_fc2")
    ln1_g, ln1_b, ln2_g, ln2_b = f("ln1_g"), f("ln1_b"), f("ln2_g"), f("ln2_b")
    b_proj, b_fc1, b_fc2 = f("b_proj"), f("b_fc1"), f("b_fc2")

    bf = ml_dtypes.bfloat16
    f8 = ml_dtypes.float8_e4m3

    def q8(w, scale):
        return np.ascontiguousarray(
            np.clip(w * scale, -240.0, 240.0).astype(f8)
        )

    w_qkv_e = ln1_g[:, None] * w_qkv
    qkv_bias = ln1_b @ w_qkv  # [2304]
    qk_bias = np.zeros((P, 2 * H), dtype=np.float32)
    for which in range(2):
        for h in range(H):
            qk_bias[0:DH, which * H + h] = S_A * qkv_bias[
                which * C + h * DH: which * C + (h + 1) * DH
            ]
    vb = qkv_bias[2 * C: 3 * C]  # v bias passes through softmax additively
    b_proj_e = b_proj + vb @ w_proj
    # head-aligned w_proj rows: block h rows 1..96 (row 0 pairs with colsum row)
    w_proj_p = np.zeros((H * P, C), dtype=np.float32)
    for h in range(H):
        w_proj_p[h * P + 1: h * P + 1 + DH, :] = w_proj[h * DH:(h + 1) * DH, :]
    w_fc1_e = ln2_g[:, None] * w_fc1
    b_fc1_e = b_fc1 + ln2_b @ w_fc1

    common = {
        "w_qkv_e": q8(w_qkv_e, S_W),
        "w_proj_p": q8(w_proj_p, S_W),
        "w_fc1_e": np.ascontiguousarray(w_fc1_e.astype(bf)),
        "w_fc2": np.ascontiguousarray(w_fc2.astype(bf)),
        "qk_bias": qk_bias,
        "b_proj_e": np.ascontiguousarray(b_proj_e.astype(bf)),
        "b_fc1_e": b_fc1_e,
        "b_fc2": np.ascontiguousarray(b_fc2.astype(bf)),
    }
    xb = x.astype(bf)
    return [dict(common, x_bf=np.ascontiguousarray(xb[i])) for i in range(8)]


def kernel(**inputs):
    if "nc" not in _CACHED:
        _CACHED["nc"] = build()
    nc = _CACHED["nc"]
    in_maps = _prep_inputs(inputs)
    res = run_bass_kernel_spmd(nc, in_maps, core_ids=list(range(8)))
    out = np.stack([res.results[i]["y"] for i in range(8)], axis=0)
    return out.astype(np.float32)
